# revision 7
# baseline (speedup 1.0000x reference)
"""Trainium2 Bass kernel for nn_CandidateSelector (gather + MLP scoring + global top-k).

Strategy (8 NeuronCores, SPMD):
  - Host packs [x | h | degree | beta] into one [200000, 322] f32 table.
  - exp_nodes is sharded 12500/core (padded to 12544 = 98*128).
  - Per core: indirect-DMA row gather (512 rows/macro-tile), PE transposes to
    feature-on-partition layout, fp32 matmuls for the MLP, scores for all
    local entries, then the GPSIMD topk instruction extracts the local
    top-256 (values + positions).
  - Host merges 8x256 candidates: sort by (score desc, entry asc) - the exact
    tie-break jax.lax.top_k uses (duplicated exp_nodes entries have bitwise
    identical scores) - takes 128, and gathers exp_nodes[idx].
  - softmax is monotonic and candidates==1.0 in eval forward, so scores alone
    determine the output.
"""

import os
import sys

import numpy as np

sys.path.insert(0, "/opt/trn_rl_repo")

N_NODES = 200000
FEAT = 256
EMB = 64
N_EXP = 100000
N_TGT = 1024
K_OUT = 128

N_CORES = 8
E_SH = N_EXP // N_CORES          # 12500
P = 128
N_TILES = (E_SH + P - 1) // P    # 98
E_PAD = N_TILES * P              # 12544
ROW = FEAT + EMB + 2             # 322 packed row (x, h, deg, beta)
MACRO = 512                      # rows per macro-tile (4 blocks of 128)

TOPK_VOCAB = 50176               # smallest legal gpsimd-topk vocab >= E_PAD
TOPK_COLS = TOPK_VOCAB // 16     # 3136
TOPK_K = 256
NEG_INF = float(np.float32(-3.0e38))

_CACHE = {}
LAST_RUN = {}


def _build_program(use_f32r: bool):
    import concourse.bacc as bacc
    import concourse.bass as bass
    import concourse.mybir as mybir
    import concourse.tile as tile
    from concourse import library_config
    from concourse.tile_rust import add_dep_helper

    f32 = mybir.dt.float32
    i32 = mybir.dt.int32
    u32 = mybir.dt.uint32
    AF = mybir.ActivationFunctionType

    nc = bacc.Bacc(
        "TRN2",
        target_bir_lowering=False,
        debug=False,
        num_devices=N_CORES,
    )

    # ---- DRAM I/O -------------------------------------------------------
    table = nc.dram_tensor("table", [N_NODES, ROW], f32, kind="ExternalInput")
    idx_d = nc.dram_tensor("idx", [P, N_TILES], i32, kind="ExternalInput")
    tgt_d = nc.dram_tensor("tgt", [P, N_TGT // P], i32, kind="ExternalInput")
    wraw_d = nc.dram_tensor("wraw", [FEAT, EMB], f32, kind="ExternalInput")
    wnum_d = nc.dram_tensor("wnum", [2, EMB], f32, kind="ExternalInput")
    w1p_d = nc.dram_tensor("w1p", [3 * EMB, EMB], f32, kind="ExternalInput")
    w1c_d = nc.dram_tensor("w1c", [EMB, EMB], f32, kind="ExternalInput")
    w2_d = nc.dram_tensor("w2", [EMB, 1], f32, kind="ExternalInput")
    bxh_d = nc.dram_tensor("bxh", [P, 1], f32, kind="ExternalInput")
    bnum_d = nc.dram_tensor("bnum", [EMB, 1], f32, kind="ExternalInput")
    b1c_d = nc.dram_tensor("b1c", [EMB, 1], f32, kind="ExternalInput")
    ident_d = nc.dram_tensor("ident", [P, P], f32, kind="ExternalInput")

    topk_out_d = nc.dram_tensor("topk_out", [16, 2 * TOPK_K // 16], u32,
                                kind="ExternalOutput")
    scores_out_d = nc.dram_tensor("scores_out", [E_PAD], f32,
                                  kind="ExternalOutput")

    mmdt = mybir.dt.float32r if use_f32r else f32

    def mv(ap):
        return ap.bitcast(mmdt) if use_f32r else ap

    with tile.TileContext(nc) as tc:
        with (
            tc.tile_pool(name="const", bufs=1) as cpool,
            tc.tile_pool(name="gather", bufs=2) as gpool,
            tc.tile_pool(name="xts", bufs=2) as xtpool,
            tc.tile_pool(name="emb", bufs=2) as epool,
            tc.tile_pool(name="score", bufs=1) as spool,
            tc.tile_pool(name="dram", bufs=1, space="DRAM") as dpool,
            tc.tile_pool(name="ps_xt0", bufs=1, space="PSUM") as pp_xt0,
            tc.tile_pool(name="ps_xt1", bufs=1, space="PSUM") as pp_xt1,
            tc.tile_pool(name="ps_xh", bufs=1, space="PSUM") as pp_xh,
            tc.tile_pool(name="ps_db", bufs=1, space="PSUM") as pp_db,
            tc.tile_pool(name="ps_en", bufs=1, space="PSUM") as pp_en,
            tc.tile_pool(name="ps_hid", bufs=1, space="PSUM") as pp_hid,
            tc.tile_pool(name="ps_sc", bufs=1, space="PSUM") as pp_sc,
        ):
            # ---- constants into SBUF ---------------------------------
            ident = cpool.tile([P, P], f32)
            nc.sync.dma_start(ident[:], ident_d[:, :])
            wraw0 = cpool.tile([P, EMB], f32)
            wraw1 = cpool.tile([P, EMB], f32)
            nc.sync.dma_start(wraw0[:], wraw_d[:P, :])
            nc.sync.dma_start(wraw1[:], wraw_d[P:, :])
            wnum = cpool.tile([2, EMB], f32)
            nc.sync.dma_start(wnum[:], wnum_d[:, :])
            w1p0 = cpool.tile([P, EMB], f32)
            w1p1 = cpool.tile([EMB, EMB], f32)
            nc.sync.dma_start(w1p0[:], w1p_d[:P, :])
            nc.sync.dma_start(w1p1[:], w1p_d[P:, :])
            w1c = cpool.tile([EMB, EMB], f32)
            nc.sync.dma_start(w1c[:], w1c_d[:, :])
            w2 = cpool.tile([EMB, 1], f32)
            nc.sync.dma_start(w2[:], w2_d[:, :])
            bxh = cpool.tile([P, 1], f32)
            nc.sync.dma_start(bxh[:], bxh_d[:, :])
            bnum = cpool.tile([EMB, 1], f32)
            nc.sync.dma_start(bnum[:], bnum_d[:, :])
            b1c = cpool.tile([EMB, 1], f32)
            nc.sync.dma_start(b1c[:], b1c_d[:, :])
            idx_sb = cpool.tile([P, N_TILES], i32)
            nc.sync.dma_start(idx_sb[:], idx_d[:, :])
            tgt_sb = cpool.tile([P, N_TGT // P], i32)
            nc.sync.dma_start(tgt_sb[:], tgt_d[:, :])

            # topk input: init everything to -inf, real scores land later.
            tk_in = cpool.tile([16, TOPK_COLS], f32)
            nc.vector.memset(tk_in[:], NEG_INF)
            tk_out = cpool.tile([16, 2 * TOPK_K // 16], u32)

            scores = spool.tile([1, E_PAD], f32)

            # ---- prologue: h_T mean -> folded bias2 ------------------
            # gather the 1024 target rows, transpose the h-slice, accumulate
            NBT = N_TGT // P  # 8
            gt = gpool.tile([P, NBT * ROW], f32, tag="G")
            for t in range(NBT):
                nc.gpsimd.indirect_dma_start(
                    out=gt[:, t * ROW:(t + 1) * ROW],
                    out_offset=None,
                    in_=table[:, :],
                    in_offset=bass.IndirectOffsetOnAxis(
                        ap=tgt_sb[:, t:t + 1], axis=0),
                )
            ps_ht = pp_sc.tile([EMB, P], f32, tag="sc")
            for t in range(NBT):
                o = t * ROW + FEAT
                nc.tensor.matmul(
                    ps_ht[:, :], lhsT=gt[:, o:o + EMB], rhs=ident[:],
                    is_transpose=True, start=(t == 0), stop=(t == NBT - 1),
                )
            hsum = cpool.tile([EMB, 1], f32)
            nc.vector.reduce_sum(out=hsum[:], in_=ps_ht[:, :],
                                 axis=mybir.AxisListType.X)
            rht = cpool.tile([EMB, 1], f32)
            nc.scalar.activation(rht[:], hsum[:], AF.Relu, scale=1.0 / N_TGT)
            ps_c1 = pp_sc.tile([EMB, 1], f32, tag="sc")
            nc.tensor.matmul(ps_c1[:, :], lhsT=w1c[:], rhs=rht[:],
                             start=True, stop=True)
            bias2 = cpool.tile([EMB, 1], f32)
            nc.vector.tensor_add(out=bias2[:], in0=b1c[:], in1=ps_c1[:, :])

            # ---- main loop -------------------------------------------
            n_macros = (E_PAD + MACRO - 1) // MACRO  # 24 full + 1 half
            for m in range(n_macros):
                base = m * MACRO
                W = min(MACRO, E_PAD - base)
                nblk = W // P

                g = gpool.tile([P, (MACRO // P) * ROW], f32, tag="G")
                t0 = base // P
                for j in range(nblk):
                    nc.gpsimd.indirect_dma_start(
                        out=g[:, j * ROW:(j + 1) * ROW],
                        out_offset=None,
                        in_=table[:, :],
                        in_offset=bass.IndirectOffsetOnAxis(
                            ap=idx_sb[:, t0 + j:t0 + j + 1], axis=0),
                    )

                ps_xt0 = pp_xt0.tile([P, MACRO], f32, tag="xt0")
                ps_xt1 = pp_xt1.tile([P, MACRO], f32, tag="xt1")
                ps_xv = pp_xh.tile([EMB, MACRO], f32, tag="xv")
                ps_h = pp_xh.tile([EMB, MACRO], f32, tag="hT")
                ps_db = pp_db.tile([2, MACRO], f32, tag="db")
                for j in range(nblk):
                    o = j * ROW
                    c = slice(j * P, (j + 1) * P)
                    nc.tensor.matmul(ps_xt0[:, c], lhsT=g[:, o:o + P],
                                     rhs=ident[:], is_transpose=True,
                                     start=True, stop=True)
                    nc.tensor.matmul(ps_xt1[:, c], lhsT=g[:, o + P:o + 2 * P],
                                     rhs=ident[:], is_transpose=True,
                                     start=True, stop=True)
                    nc.tensor.matmul(ps_h[:, c],
                                     lhsT=g[:, o + FEAT:o + FEAT + EMB],
                                     rhs=ident[:], is_transpose=True,
                                     start=True, stop=True)
                    nc.tensor.matmul(ps_db[:, c],
                                     lhsT=g[:, o + FEAT + EMB:o + ROW],
                                     rhs=ident[:], is_transpose=True,
                                     start=True, stop=True)

                xt0 = xtpool.tile([P, MACRO], f32, tag="xt0sb")
                xt1 = xtpool.tile([P, MACRO], f32, tag="xt1sb")
                db = epool.tile([2, MACRO], f32, tag="db")
                nc.vector.tensor_copy(xt0[:, :W], ps_xt0[:, :W])
                nc.scalar.copy(xt1[:, :W], ps_xt1[:, :W])
                nc.vector.tensor_copy(db[:, :W], ps_db[:, :W])

                # x_v^T = W_raw^T @ x^T  (accumulated over the 2 K-chunks)
                nc.tensor.matmul(ps_xv[:, :W], lhsT=mv(wraw0[:]),
                                 rhs=mv(xt0[:, :W]), start=True, stop=False)
                nc.tensor.matmul(ps_xv[:, :W], lhsT=mv(wraw1[:]),
                                 rhs=mv(xt1[:, :W]), start=False, stop=True)

                # emb_a = relu([x_v^T ; h_v^T] + [b_raw ; 0])
                emb_a = epool.tile([P, MACRO], f32, tag="emba")
                nc.scalar.activation(emb_a[:EMB, :W], ps_xv[:, :W], AF.Relu,
                                     bias=bxh[:EMB])
                nc.scalar.activation(emb_a[EMB:, :W], ps_h[:, :W], AF.Relu,
                                     bias=bxh[EMB:])

                # emb_b = relu(W_num^T @ [deg;beta] + b_num)
                ps_en = pp_en.tile([EMB, MACRO], f32, tag="en")
                nc.tensor.matmul(ps_en[:, :W], lhsT=mv(wnum[:]),
                                 rhs=mv(db[:, :W]), start=True, stop=True)
                emb_b = epool.tile([EMB, MACRO], f32, tag="embb")
                nc.scalar.activation(emb_b[:, :W], ps_en[:, :W], AF.Relu,
                                     bias=bnum[:])

                # hidden = relu(W1'^T @ emb + b1 + c1)
                ps_hid = pp_hid.tile([EMB, MACRO], f32, tag="hid")
                nc.tensor.matmul(ps_hid[:, :W], lhsT=mv(w1p0[:]),
                                 rhs=mv(emb_a[:, :W]), start=True, stop=False)
                nc.tensor.matmul(ps_hid[:, :W], lhsT=mv(w1p1[:]),
                                 rhs=mv(emb_b[:, :W]), start=False, stop=True)
                hid = epool.tile([EMB, MACRO], f32, tag="hidsb")
                nc.scalar.activation(hid[:, :W], ps_hid[:, :W], AF.Relu,
                                     bias=bias2[:])

                # scores = W2^T @ hidden
                ps_sc = pp_sc.tile([1, MACRO], f32, tag="sc")
                nc.tensor.matmul(ps_sc[:, :W], lhsT=mv(w2[:]),
                                 rhs=mv(hid[:, :W]), start=True, stop=True)
                nc.vector.tensor_copy(scores[:, base:base + W], ps_sc[:, :W])

            # ---- epilogue: local top-256 ------------------------------
            nc.vector.memset(scores[:, E_SH:E_PAD], NEG_INF)
            nc.sync.dma_start(out=scores_out_d[:], in_=scores[:, :])
            sc_b = dpool.tile([E_PAD], f32)
            nc.sync.dma_start(out=sc_b[:], in_=scores[:, :])
            nc.sync.dma_start(out=tk_in[: E_PAD // TOPK_COLS, :], in_=sc_b[:])

            lib = nc.gpsimd.load_library(library_config.topk)
            # nc.gpsimd.topk(), minus its pre-Tile SBTensorHandle assert
            import concourse.bass_isa as bass_isa
            tk = nc.gpsimd.add_instruction(
                bass_isa.InstTopk(
                    name=f"I-{nc.next_id()}",
                    ins=[nc.gpsimd.lower_ap(tk_in[:], for_isa=True)],
                    outs=[nc.gpsimd.lower_ap(tk_out[:], for_isa=True)],
                    _tokens=1,
                    _n=TOPK_VOCAB,
                    _k=TOPK_K,
                )
            )
            add_dep_helper(tk.ins, lib.ins, sync=True, reason="lib before topk")
            nc.sync.dma_start(out=topk_out_d[:, :], in_=tk_out[:])

    nc.compile()
    return nc


def _get_program():
    use_f32r = os.environ.get("KERNEL_F32R", "1") == "1"
    key = ("prog", use_f32r)
    if key not in _CACHE:
        _CACHE[key] = _build_program(use_f32r)
    return _CACHE[key]


def kernel(x, h, degree, beta, exp_nodes, idx_targets,
           W_raw, b_raw, W_num, b_num, W1, b1, W2, b2,
           temperature, epsilon, **_unused):
    from concourse.bass_utils import run_bass_kernel_spmd

    x = np.asarray(x, np.float32)
    h = np.asarray(h, np.float32)
    degree = np.asarray(degree, np.float32)
    beta = np.asarray(beta, np.float32)
    exp_nodes = np.asarray(exp_nodes)
    idx_targets = np.asarray(idx_targets)

    table = np.concatenate(
        [x, h, degree[:, None], beta[:, None]], axis=1
    ).astype(np.float32)
    if not table.flags.c_contiguous:
        table = np.ascontiguousarray(table)

    # per-core index shards, padded to 12544, laid out [128, 98] partition-major
    idx_maps = []
    for c in range(N_CORES):
        sh = np.zeros(E_PAD, np.int32)
        sh[:E_SH] = exp_nodes[c * E_SH:(c + 1) * E_SH].astype(np.int32)
        idx_maps.append(np.ascontiguousarray(sh.reshape(N_TILES, P).T))

    tgt = np.ascontiguousarray(
        idx_targets.astype(np.int32).reshape(N_TGT // P, P).T)

    w1p = np.concatenate([W1[:2 * EMB], W1[3 * EMB:]]).astype(np.float32)
    w1c = np.ascontiguousarray(W1[2 * EMB:3 * EMB].astype(np.float32))
    bxh = np.zeros((P, 1), np.float32)
    bxh[:EMB, 0] = np.asarray(b_raw, np.float32)

    common = {
        "table": table,
        "tgt": tgt,
        "wraw": np.ascontiguousarray(W_raw, dtype=np.float32),
        "wnum": np.ascontiguousarray(W_num, dtype=np.float32),
        "w1p": np.ascontiguousarray(w1p),
        "w1c": w1c,
        "w2": np.ascontiguousarray(np.asarray(W2, np.float32).reshape(EMB, 1)),
        "bxh": bxh,
        "bnum": np.asarray(b_num, np.float32).reshape(EMB, 1).copy(),
        "b1c": np.asarray(b1, np.float32).reshape(EMB, 1).copy(),
        "ident": np.eye(P, dtype=np.float32),
    }
    in_maps = [dict(common, idx=idx_maps[c]) for c in range(N_CORES)]

    nc = _get_program()
    res = run_bass_kernel_spmd(
        nc, in_maps, list(range(N_CORES)),
        trace=os.environ.get("KERNEL_TRACE", "0") == "1",
    )
    LAST_RUN["exec_time_ns"] = res.exec_time_ns
    LAST_RUN["mean_exec_time_ns"] = res.mean_exec_time_ns
    LAST_RUN["results"] = res.results

    # ---- host merge: 8x256 candidates -> exact ordered top-128 ----------
    vals_all = []
    ents_all = []
    for c in range(N_CORES):
        tk = res.results[c]["topk_out"]
        vals = tk[:, :TOPK_K // 16].reshape(-1).view(np.float32).copy()
        slots = tk[:, TOPK_K // 16:].reshape(-1).astype(np.int64)
        if not (slots < E_SH).all():
            keep = slots < E_SH  # drop any padding slots (shouldn't happen)
            vals, slots = vals[keep], slots[keep]
        vals_all.append(vals)
        ents_all.append(c * E_SH + slots)
    vals_all = np.concatenate(vals_all)
    ents_all = np.concatenate(ents_all)

    order = np.lexsort((ents_all, -vals_all))[:K_OUT]
    idx128 = ents_all[order]

    candidates = np.ones(K_OUT, np.float32)
    cand_indices = exp_nodes[idx128]
    return candidates, cand_indices


# revision 9
# speedup vs baseline: 1.0393x; 1.0393x over previous
"""Trainium2 Bass kernel for nn_CandidateSelector (gather + MLP scoring + global top-k).

Strategy (8 NeuronCores, SPMD):
  - Host packs [x | h | degree | beta] into one [200000, 322] f32 table.
  - exp_nodes is sharded 12500/core (padded to 12544 = 98*128).
  - Per core: indirect-DMA row gather (512 rows/macro-tile), PE transposes to
    feature-on-partition layout, fp32 matmuls for the MLP, scores for all
    local entries, then the GPSIMD topk instruction extracts the local
    top-256 (values + positions).
  - Host merges 8x256 candidates: sort by (score desc, entry asc) - the exact
    tie-break jax.lax.top_k uses (duplicated exp_nodes entries have bitwise
    identical scores) - takes 128, and gathers exp_nodes[idx].
  - softmax is monotonic and candidates==1.0 in eval forward, so scores alone
    determine the output.
"""

import os
import sys

import numpy as np

sys.path.insert(0, "/opt/trn_rl_repo")

N_NODES = 200000
FEAT = 256
EMB = 64
N_EXP = 100000
N_TGT = 1024
K_OUT = 128

N_CORES = 8
E_SH = N_EXP // N_CORES          # 12500
P = 128
N_TILES = (E_SH + P - 1) // P    # 98
E_PAD = N_TILES * P              # 12544
ROW = FEAT + EMB + 2             # 322 packed row (x, h, deg, beta)
MACRO = 512                      # rows per macro-tile (4 blocks of 128)

TOPK_VOCAB = 50176               # smallest legal gpsimd-topk vocab >= E_PAD
TOPK_COLS = TOPK_VOCAB // 16     # 3136
TOPK_K = 256
NEG_INF = float(np.float32(-3.0e38))

_CACHE = {}
LAST_RUN = {}


def _build_program(use_f32r: bool):
    import concourse.bacc as bacc
    import concourse.bass as bass
    import concourse.mybir as mybir
    import concourse.tile as tile
    from concourse import library_config
    from concourse.tile_rust import add_dep_helper

    f32 = mybir.dt.float32
    i32 = mybir.dt.int32
    u32 = mybir.dt.uint32
    AF = mybir.ActivationFunctionType

    nc = bacc.Bacc(
        "TRN2",
        target_bir_lowering=False,
        debug=False,
        num_devices=N_CORES,
    )

    # ---- DRAM I/O -------------------------------------------------------
    table = nc.dram_tensor("table", [N_NODES, ROW], f32, kind="ExternalInput")
    idx_d = nc.dram_tensor("idx", [P, N_TILES], i32, kind="ExternalInput")
    tgt_d = nc.dram_tensor("tgt", [P, N_TGT // P], i32, kind="ExternalInput")
    wraw_d = nc.dram_tensor("wraw", [FEAT, EMB], f32, kind="ExternalInput")
    wnum_d = nc.dram_tensor("wnum", [2, EMB], f32, kind="ExternalInput")
    w1p_d = nc.dram_tensor("w1p", [3 * EMB, EMB], f32, kind="ExternalInput")
    w1c_d = nc.dram_tensor("w1c", [EMB, EMB], f32, kind="ExternalInput")
    w2_d = nc.dram_tensor("w2", [EMB, 1], f32, kind="ExternalInput")
    bxh_d = nc.dram_tensor("bxh", [P, 1], f32, kind="ExternalInput")
    bnum_d = nc.dram_tensor("bnum", [EMB, 1], f32, kind="ExternalInput")
    b1c_d = nc.dram_tensor("b1c", [EMB, 1], f32, kind="ExternalInput")
    ident_d = nc.dram_tensor("ident", [P, P], f32, kind="ExternalInput")

    topk_out_d = nc.dram_tensor("topk_out", [16, 2 * TOPK_K // 16], u32,
                                kind="ExternalOutput")
    scores_out_d = nc.dram_tensor("scores_out", [E_PAD], f32,
                                  kind="ExternalOutput")

    mmdt = mybir.dt.float32r if use_f32r else f32

    def mv(ap):
        return ap.bitcast(mmdt) if use_f32r else ap

    with tile.TileContext(nc) as tc:
        with (
            tc.tile_pool(name="const", bufs=1) as cpool,
            tc.tile_pool(name="gather", bufs=3) as gpool,
            tc.tile_pool(name="xts", bufs=2) as xtpool,
            tc.tile_pool(name="emb", bufs=2) as epool,
            tc.tile_pool(name="score", bufs=1) as spool,
            tc.tile_pool(name="dram", bufs=1, space="DRAM") as dpool,
            tc.tile_pool(name="ps_xt0", bufs=1, space="PSUM") as pp_xt0,
            tc.tile_pool(name="ps_xt1", bufs=1, space="PSUM") as pp_xt1,
            tc.tile_pool(name="ps_xh", bufs=1, space="PSUM") as pp_xh,
            tc.tile_pool(name="ps_db", bufs=1, space="PSUM") as pp_db,
            tc.tile_pool(name="ps_en", bufs=1, space="PSUM") as pp_en,
            tc.tile_pool(name="ps_hid", bufs=1, space="PSUM") as pp_hid,
            tc.tile_pool(name="ps_sc", bufs=1, space="PSUM") as pp_sc,
        ):
            # ---- constants into SBUF ---------------------------------
            ident = cpool.tile([P, P], f32)
            nc.sync.dma_start(ident[:], ident_d[:, :])
            wraw0 = cpool.tile([P, EMB], f32)
            wraw1 = cpool.tile([P, EMB], f32)
            nc.sync.dma_start(wraw0[:], wraw_d[:P, :])
            nc.sync.dma_start(wraw1[:], wraw_d[P:, :])
            wnum = cpool.tile([2, EMB], f32)
            nc.sync.dma_start(wnum[:], wnum_d[:, :])
            w1p0 = cpool.tile([P, EMB], f32)
            w1p1 = cpool.tile([EMB, EMB], f32)
            nc.sync.dma_start(w1p0[:], w1p_d[:P, :])
            nc.sync.dma_start(w1p1[:], w1p_d[P:, :])
            w1c = cpool.tile([EMB, EMB], f32)
            nc.sync.dma_start(w1c[:], w1c_d[:, :])
            w2 = cpool.tile([EMB, 1], f32)
            nc.sync.dma_start(w2[:], w2_d[:, :])
            bxh = cpool.tile([P, 1], f32)
            nc.sync.dma_start(bxh[:], bxh_d[:, :])
            bnum = cpool.tile([EMB, 1], f32)
            nc.sync.dma_start(bnum[:], bnum_d[:, :])
            b1c = cpool.tile([EMB, 1], f32)
            nc.sync.dma_start(b1c[:], b1c_d[:, :])
            idx_sb = cpool.tile([P, N_TILES], i32)
            nc.sync.dma_start(idx_sb[:], idx_d[:, :])
            tgt_sb = cpool.tile([P, N_TGT // P], i32)
            nc.sync.dma_start(tgt_sb[:], tgt_d[:, :])

            # topk input: init everything to -inf, real scores land later.
            tk_in = cpool.tile([16, TOPK_COLS], f32)
            nc.vector.memset(tk_in[:], NEG_INF)
            tk_out = cpool.tile([16, 2 * TOPK_K // 16], u32)

            scores = spool.tile([1, E_PAD], f32)

            lib = nc.gpsimd.load_library(library_config.topk)

            # ---- prologue: h_T mean -> folded bias2 ------------------
            # gather the 1024 target rows, transpose the h-slice, accumulate
            NBT = N_TGT // P  # 8
            gt = gpool.tile([P, NBT * ROW], f32, tag="G")
            for t in range(NBT):
                nc.gpsimd.indirect_dma_start(
                    out=gt[:, t * ROW:(t + 1) * ROW],
                    out_offset=None,
                    in_=table[:, :],
                    in_offset=bass.IndirectOffsetOnAxis(
                        ap=tgt_sb[:, t:t + 1], axis=0),
                )
            ps_ht = pp_sc.tile([EMB, P], f32, tag="sc")
            for t in range(NBT):
                o = t * ROW + FEAT
                nc.tensor.matmul(
                    ps_ht[:, :], lhsT=gt[:, o:o + EMB], rhs=ident[:],
                    is_transpose=True, start=(t == 0), stop=(t == NBT - 1),
                )
            hsum = cpool.tile([EMB, 1], f32)
            nc.vector.reduce_sum(out=hsum[:], in_=ps_ht[:, :],
                                 axis=mybir.AxisListType.X)
            rht = cpool.tile([EMB, 1], f32)
            nc.scalar.activation(rht[:], hsum[:], AF.Relu, scale=1.0 / N_TGT)
            ps_c1 = pp_sc.tile([EMB, 1], f32, tag="sc")
            nc.tensor.matmul(ps_c1[:, :], lhsT=w1c[:], rhs=rht[:],
                             start=True, stop=True)
            bias2 = cpool.tile([EMB, 1], f32)
            nc.vector.tensor_add(out=bias2[:], in0=b1c[:], in1=ps_c1[:, :])

            # ---- main loop -------------------------------------------
            n_macros = (E_PAD + MACRO - 1) // MACRO  # 24 full + 1 half
            for m in range(n_macros):
                base = m * MACRO
                W = min(MACRO, E_PAD - base)
                nblk = W // P

                g = gpool.tile([P, (MACRO // P) * ROW], f32, tag="G")
                t0 = base // P
                for j in range(nblk):
                    nc.gpsimd.indirect_dma_start(
                        out=g[:, j * ROW:(j + 1) * ROW],
                        out_offset=None,
                        in_=table[:, :],
                        in_offset=bass.IndirectOffsetOnAxis(
                            ap=idx_sb[:, t0 + j:t0 + j + 1], axis=0),
                    )

                ps_xt0 = pp_xt0.tile([P, MACRO], f32, tag="xt0")
                ps_xt1 = pp_xt1.tile([P, MACRO], f32, tag="xt1")
                ps_xv = pp_xh.tile([EMB, MACRO], f32, tag="xv", bufs=2)
                ps_h = pp_db.tile([EMB + 2, MACRO], f32, tag="hdb")
                for j in range(nblk):
                    o = j * ROW
                    c = slice(j * P, (j + 1) * P)
                    nc.tensor.matmul(ps_xt0[:, c], lhsT=g[:, o:o + P],
                                     rhs=ident[:], is_transpose=True,
                                     start=True, stop=True)
                    nc.tensor.matmul(ps_xt1[:, c], lhsT=g[:, o + P:o + 2 * P],
                                     rhs=ident[:], is_transpose=True,
                                     start=True, stop=True)
                    nc.tensor.matmul(ps_h[:, c],
                                     lhsT=g[:, o + FEAT:o + ROW],
                                     rhs=ident[:], is_transpose=True,
                                     start=True, stop=True)

                xt0 = xtpool.tile([P, MACRO], f32, tag="xt0sb")
                xt1 = xtpool.tile([P, MACRO], f32, tag="xt1sb")
                db = epool.tile([2, MACRO], f32, tag="db")
                nc.vector.tensor_copy(xt0[:, :W], ps_xt0[:, :W])
                nc.scalar.copy(xt1[:, :W], ps_xt1[:, :W])
                nc.vector.tensor_copy(db[:, :W], ps_h[EMB:, :W])

                # x_v^T = W_raw^T @ x^T  (accumulated over the 2 K-chunks)
                nc.tensor.matmul(ps_xv[:, :W], lhsT=mv(wraw0[:]),
                                 rhs=mv(xt0[:, :W]), start=True, stop=False)
                nc.tensor.matmul(ps_xv[:, :W], lhsT=mv(wraw1[:]),
                                 rhs=mv(xt1[:, :W]), start=False, stop=True)

                # emb_a = relu([x_v^T ; h_v^T] + [b_raw ; 0])
                emb_a = epool.tile([P, MACRO], f32, tag="emba")
                nc.scalar.activation(emb_a[:EMB, :W], ps_xv[:, :W], AF.Relu,
                                     bias=bxh[:EMB])
                nc.scalar.activation(emb_a[EMB:, :W], ps_h[:EMB, :W], AF.Relu,
                                     bias=bxh[EMB:])

                # emb_b = relu(W_num^T @ [deg;beta] + b_num)
                ps_en = pp_en.tile([EMB, MACRO], f32, tag="en")
                nc.tensor.matmul(ps_en[:, :W], lhsT=mv(wnum[:]),
                                 rhs=mv(db[:, :W]), start=True, stop=True)
                emb_b = epool.tile([EMB, MACRO], f32, tag="embb")
                nc.scalar.activation(emb_b[:, :W], ps_en[:, :W], AF.Relu,
                                     bias=bnum[:])

                # hidden = relu(W1'^T @ emb + b1 + c1)
                ps_hid = pp_hid.tile([EMB, MACRO], f32, tag="hid")
                nc.tensor.matmul(ps_hid[:, :W], lhsT=mv(w1p0[:]),
                                 rhs=mv(emb_a[:, :W]), start=True, stop=False)
                nc.tensor.matmul(ps_hid[:, :W], lhsT=mv(w1p1[:]),
                                 rhs=mv(emb_b[:, :W]), start=False, stop=True)
                hid = epool.tile([EMB, MACRO], f32, tag="hidsb")
                nc.scalar.activation(hid[:, :W], ps_hid[:, :W], AF.Relu,
                                     bias=bias2[:])

                # scores = W2^T @ hidden
                ps_sc = pp_sc.tile([1, MACRO], f32, tag="sc")
                nc.tensor.matmul(ps_sc[:, :W], lhsT=mv(w2[:]),
                                 rhs=mv(hid[:, :W]), start=True, stop=True)
                nc.vector.tensor_copy(scores[:, base:base + W], ps_sc[:, :W])

            # ---- epilogue: local top-256 ------------------------------
            nc.vector.memset(scores[:, E_SH:E_PAD], NEG_INF)
            nc.sync.dma_start(out=scores_out_d[:], in_=scores[:, :])
            sc_b = dpool.tile([E_PAD], f32)
            nc.sync.dma_start(out=sc_b[:], in_=scores[:, :])
            nc.sync.dma_start(out=tk_in[: E_PAD // TOPK_COLS, :], in_=sc_b[:])

            # nc.gpsimd.topk(), minus its pre-Tile SBTensorHandle assert
            import concourse.bass_isa as bass_isa
            tk = nc.gpsimd.add_instruction(
                bass_isa.InstTopk(
                    name=f"I-{nc.next_id()}",
                    ins=[nc.gpsimd.lower_ap(tk_in[:], for_isa=True)],
                    outs=[nc.gpsimd.lower_ap(tk_out[:], for_isa=True)],
                    _tokens=1,
                    _n=TOPK_VOCAB,
                    _k=TOPK_K,
                )
            )
            add_dep_helper(tk.ins, lib.ins, sync=True, reason="lib before topk")
            nc.sync.dma_start(out=topk_out_d[:, :], in_=tk_out[:])

    nc.compile()
    return nc


def _get_program():
    use_f32r = os.environ.get("KERNEL_F32R", "1") == "1"
    key = ("prog", use_f32r)
    if key not in _CACHE:
        _CACHE[key] = _build_program(use_f32r)
    return _CACHE[key]


def kernel(x, h, degree, beta, exp_nodes, idx_targets,
           W_raw, b_raw, W_num, b_num, W1, b1, W2, b2,
           temperature, epsilon, **_unused):
    from concourse.bass_utils import run_bass_kernel_spmd

    x = np.asarray(x, np.float32)
    h = np.asarray(h, np.float32)
    degree = np.asarray(degree, np.float32)
    beta = np.asarray(beta, np.float32)
    exp_nodes = np.asarray(exp_nodes)
    idx_targets = np.asarray(idx_targets)

    table = np.concatenate(
        [x, h, degree[:, None], beta[:, None]], axis=1
    ).astype(np.float32)
    if not table.flags.c_contiguous:
        table = np.ascontiguousarray(table)

    # per-core index shards, padded to 12544, laid out [128, 98] partition-major
    idx_maps = []
    for c in range(N_CORES):
        sh = np.zeros(E_PAD, np.int32)
        sh[:E_SH] = exp_nodes[c * E_SH:(c + 1) * E_SH].astype(np.int32)
        idx_maps.append(np.ascontiguousarray(sh.reshape(N_TILES, P).T))

    tgt = np.ascontiguousarray(
        idx_targets.astype(np.int32).reshape(N_TGT // P, P).T)

    w1p = np.concatenate([W1[:2 * EMB], W1[3 * EMB:]]).astype(np.float32)
    w1c = np.ascontiguousarray(W1[2 * EMB:3 * EMB].astype(np.float32))
    bxh = np.zeros((P, 1), np.float32)
    bxh[:EMB, 0] = np.asarray(b_raw, np.float32)

    common = {
        "table": table,
        "tgt": tgt,
        "wraw": np.ascontiguousarray(W_raw, dtype=np.float32),
        "wnum": np.ascontiguousarray(W_num, dtype=np.float32),
        "w1p": np.ascontiguousarray(w1p),
        "w1c": w1c,
        "w2": np.ascontiguousarray(np.asarray(W2, np.float32).reshape(EMB, 1)),
        "bxh": bxh,
        "bnum": np.asarray(b_num, np.float32).reshape(EMB, 1).copy(),
        "b1c": np.asarray(b1, np.float32).reshape(EMB, 1).copy(),
        "ident": np.eye(P, dtype=np.float32),
    }
    in_maps = [dict(common, idx=idx_maps[c]) for c in range(N_CORES)]

    nc = _get_program()
    res = run_bass_kernel_spmd(
        nc, in_maps, list(range(N_CORES)),
        trace=os.environ.get("KERNEL_TRACE", "0") == "1",
    )
    LAST_RUN["exec_time_ns"] = res.exec_time_ns
    LAST_RUN["mean_exec_time_ns"] = res.mean_exec_time_ns
    LAST_RUN["results"] = res.results

    # ---- host merge: 8x256 candidates -> exact ordered top-128 ----------
    vals_all = []
    ents_all = []
    for c in range(N_CORES):
        tk = res.results[c]["topk_out"]
        vals = tk[:, :TOPK_K // 16].reshape(-1).view(np.float32).copy()
        slots = tk[:, TOPK_K // 16:].reshape(-1).astype(np.int64)
        if not (slots < E_SH).all():
            keep = slots < E_SH  # drop any padding slots (shouldn't happen)
            vals, slots = vals[keep], slots[keep]
        vals_all.append(vals)
        ents_all.append(c * E_SH + slots)
    vals_all = np.concatenate(vals_all)
    ents_all = np.concatenate(ents_all)

    order = np.lexsort((ents_all, -vals_all))[:K_OUT]
    idx128 = ents_all[order]

    candidates = np.ones(K_OUT, np.float32)
    cand_indices = exp_nodes[idx128]
    return candidates, cand_indices
